# revision 35
# baseline (speedup 1.0000x reference)
"""AdaLN cross-attention + FFN block on 8 TRN2 NeuronCores.

Sharding: 8 cores = 4 batches x 2 L-halves (512 rows each). No collectives:
K/V projections are duplicated across the pair of cores sharing a batch
(~14% extra FLOPs), everything else splits cleanly along L.

Layout: the whole kernel runs TRANSPOSED — activations are [C, L] with the
channel dim on partitions. This makes every matmul natural (contraction dim
on partitions), makes the AdaLN scale/shift/gamma per-partition broadcasts,
and costs zero on-device transposes. The host supplies x^T, context^T,
exp(bias)^T (per-head [m, l]) and pre-transposed weights; the output comes
back as out^T and is transposed on host.

dtypes: bf16 matmul inputs for QKV/scores/attention/FFN (f32 PSUM
accumulation everywhere), float32r for the output projection, f32 for
LayerNorm statistics and residuals.

The emission order is software-pipelined (PE executes its queue in order):
the attention loop runs with a two-iteration skew — scores for head-pair i,
attention*V for pair i-1, and normalization for pair i-2 are emitted
together — so no engine ever waits on another's freshest output.
"""
import sys
if "/opt/trn_rl_repo" not in sys.path:
    sys.path.insert(0, "/opt/trn_rl_repo")

import numpy as np
import ml_dtypes

import concourse.bass as bass
import concourse.mybir as mybir
import concourse.tile as tile
from concourse import bacc
from concourse.bass_utils import run_bass_kernel_spmd

B, L, LC, C, H, HD = 4, 1024, 1024, 1024, 16, 64
P = 128
LH = 512                 # L rows per core
CT = C // P              # 8
MT = LC // P             # 8
E = 4 * C                # 4096
ET = E // P              # 32
SCALE = 0.25 / (HD ** 0.5)
EPS = 1e-5

F32 = mybir.dt.float32
F32R = mybir.dt.float32r
BF16 = mybir.dt.bfloat16
AF = mybir.ActivationFunctionType
ALU = mybir.AluOpType

NCORES = 8


def build():
    nc = bacc.Bacc("TRN2", target_bir_lowering=False, debug=False, num_devices=NCORES)

    xT_d = nc.declare_dram_parameter("xT", [C, LH], F32, isOutput=False)
    ctxT_d = nc.declare_dram_parameter("ctxT", [C, LC], BF16, isOutput=False)
    biasT_d = nc.declare_dram_parameter("biasT", [H, LC, LH], BF16, isOutput=False)
    wqT_d = nc.declare_dram_parameter("wqT", [P, CT, CT, P], BF16, isOutput=False)
    wkT_d = nc.declare_dram_parameter("wkT", [P, CT, CT, P], BF16, isOutput=False)
    wvT_d = nc.declare_dram_parameter("wvT", [C, C], BF16, isOutput=False)
    woT_d = nc.declare_dram_parameter("woT", [P, CT, CT, P], F32R, isOutput=False)
    w1T_d = nc.declare_dram_parameter("w1T", [P, ET, CT, P], BF16, isOutput=False)
    w2T_d = nc.declare_dram_parameter("w2T", [P, CT, 2, 16, P], BF16, isOutput=False)
    bo_d = nc.declare_dram_parameter("bo", [P, CT], F32, isOutput=False)
    b1_d = nc.declare_dram_parameter("b1", [P, ET], F32, isOutput=False)
    b2_d = nc.declare_dram_parameter("b2", [P, CT], F32, isOutput=False)
    cond_d = nc.declare_dram_parameter("cond", [P, 6, CT], F32, isOutput=False)
    ada_d = nc.declare_dram_parameter("ada", [P, 6, CT], F32, isOutput=False)
    outT_d = nc.declare_dram_parameter("outT", [C, LH], F32, isOutput=True)

    with tile.TileContext(nc) as tc:
        with (
            tc.tile_pool(name="cn", bufs=1) as cn,
            tc.tile_pool(name="p1", bufs=1) as p1,
            tc.tile_pool(name="p2", bufs=2) as p2,
            tc.tile_pool(name="p3", bufs=3) as p3,
            tc.tile_pool(name="p4", bufs=4) as p4,
            tc.tile_pool(name="pp", bufs=4) as pp,
            tc.tile_pool(name="rw2", bufs=2) as rw2,
            tc.tile_pool(name="psS", bufs=3, space="PSUM") as psS,      # [P,1024] universal
            tc.tile_pool(name="psA", bufs=2, space="PSUM") as psA,      # [P,512] av/stats/bcast
        ):
            # ---------- constants / params ----------
            ones_f = cn.tile([P, 1], F32, tag="ones_f")
            nc.vector.memset(ones_f[:], 1.0)
            ones_col = cn.tile([P, 1], BF16, tag="ones_col")        # lhsT [K=P, M=1]
            nc.scalar.copy(ones_col[:], ones_f[:])
            ones_col32 = cn.tile([P, 1], F32, tag="ones_col32")
            nc.vector.tensor_copy(ones_col32[:], ones_f[:])
            ones_rf = cn.tile([1, P], F32, tag="ones_rf")
            nc.vector.memset(ones_rf[:], 1.0)
            ones_row = cn.tile([1, P], F32R, tag="ones_row")        # lhsT [K=1, M=P]
            nc.scalar.copy(ones_row[:], ones_rf[:])
            eps_t = cn.tile([1, 1], F32, tag="eps")
            nc.vector.memset(eps_t[:], EPS)

            # ---------- big persistent tensors ----------
            ctxT = p1.tile([P, CT, LC], BF16, tag="bigA")           # shared slot with hT
            wvS = p1.tile([P, CT, 512], BF16, tag="wvS")            # half of Wv^T
            xT = p1.tile([P, CT, LH], F32, tag="xT")
            qT = p1.tile([P, CT, LH], BF16, tag="qT")
            vaug = p1.tile([P, MT, H, HD + 1], BF16, tag="vaug")
            nc.vector.memset(vaug[:, :, :, HD:HD + 1], 1.0)         # softmax-denominator ones column

            # ---------- LN pieces ----------
            def ln_stats_ct(src, xsum, ssum, ct, f32_stats=False):
                if f32_stats:
                    # direct f32 matmuls (4 cyc/row) — PE is idle here anyway and
                    # this kills the DMA->cast->matmul staircase at startup
                    sq = p2.tile([P, LH], F32, tag="kdt")
                    nc.scalar.activation(sq[:], src[:, ct, :], AF.Square, bias=0.0, scale=1.0)
                    nc.tensor.matmul(xsum[:], ones_col32[:], src[:, ct, :],
                                     start=(ct == 0), stop=(ct == CT - 1))
                    nc.tensor.matmul(ssum[:], ones_col32[:], sq[:],
                                     start=(ct == 0), stop=(ct == CT - 1))
                    return
                xr = p4.tile([P, LH], BF16, tag="tmpA")
                nc.vector.tensor_copy(xr[:], src[:, ct, :])
                sq = p4.tile([P, LH], BF16, tag="tmpB")
                nc.scalar.activation(sq[:], src[:, ct, :], AF.Square, bias=0.0, scale=1.0)
                nc.tensor.matmul(xsum[:], ones_col[:], xr[:], start=(ct == 0), stop=(ct == CT - 1))
                nc.tensor.matmul(ssum[:], ones_col[:], sq[:], start=(ct == 0), stop=(ct == CT - 1))

            def ln_rows(xsum, ssum):
                mu = cn.tile([1, LH], F32, tag="mu")
                nc.vector.tensor_scalar_mul(mu[:], xsum[:], 1.0 / C)
                ex2 = p3.tile([1, LH], F32, tag="rowtmp")
                nc.vector.tensor_scalar_mul(ex2[:], ssum[:], 1.0 / C)
                mu2 = p3.tile([1, LH], F32, tag="rowtmp")
                nc.vector.tensor_tensor(mu2[:], mu[:], mu[:], ALU.mult)
                var = p3.tile([1, LH], F32, tag="rowtmp")
                nc.vector.tensor_tensor(var[:], ex2[:], mu2[:], ALU.subtract)
                sd = p3.tile([1, LH], F32, tag="rowtmp")
                nc.scalar.activation(sd[:], var[:], AF.Sqrt, bias=eps_t[:, 0:1], scale=1.0)
                rstd = cn.tile([1, LH], F32, tag="rstd")
                nc.vector.reciprocal_approx_fast(rstd[:], sd[:])
                rstd_r = cn.tile([1, LH], F32R, tag="rstd_r")
                nc.scalar.copy(rstd_r[:], rstd[:])
                nmr = p3.tile([1, LH], F32, tag="rowtmp")
                nc.vector.tensor_tensor(nmr[:], mu[:], rstd[:], ALU.mult)
                nmr_r = cn.tile([1, LH], F32R, tag="nmr_r")
                nc.scalar.mul(nmr_r[:], nmr[:], -1.0)               # -(mu*rstd)
                return rstd_r, nmr_r

            def ln_bc(rstd_r, nmr_r):
                # broadcast rows across partitions via K=1 matmul, then park in
                # SBUF so the PSUM banks free up immediately
                bc_rp = psA.tile([P, LH], F32, tag="avp")
                nc.tensor.matmul(bc_rp[:], ones_row[:], rstd_r[:], start=True, stop=True)
                bc_r = rw2.tile([P, LH], F32, tag="bcs")
                nc.scalar.copy(bc_r[:], bc_rp[:])
                bc_np = psA.tile([P, LH], F32, tag="avp")
                nc.tensor.matmul(bc_np[:], ones_row[:], nmr_r[:], start=True, stop=True)
                bc_n = rw2.tile([P, LH], F32, tag="bcs")
                nc.scalar.copy(bc_n[:], bc_np[:])
                return bc_r, bc_n

            def ln_apply(src, bc_r, bc_n, sc_col, sh_idx, out_mod):
                for ct in range(CT):
                    t1 = p4.tile([P, LH], F32, tag="tmpA")
                    nc.vector.tensor_tensor(t1[:], src[:, ct, :], bc_r[:], ALU.mult)
                    t2 = p4.tile([P, LH], F32, tag="tmpB")
                    nc.vector.tensor_tensor(t2[:], t1[:], bc_n[:], ALU.add)
                    nc.scalar.activation(out_mod[:, ct, :], t2[:], AF.Identity,
                                         bias=g_t[:, sh_idx, ct:ct + 1],
                                         scale=sc_col[:, ct:ct + 1])

            # LN1 stats: xT arrives per-c-tile so the first matmuls start early
            xsum1 = psA.tile([1, LH], F32, tag="avp")
            ssum1 = psA.tile([1, LH], F32, tag="avp")
            for ct in range(CT):
                nc.sync.dma_start(xT[:, ct, :], xT_d[ct * P:(ct + 1) * P, :])
                ln_stats_ct(xT, xsum1, ssum1, ct)
            r1, n1 = ln_rows(xsum1, ssum1)
            # context + Wv stream in AFTER xT so LN1 isn't starved of HBM bandwidth
            nc.gpsimd.dma_start(ctxT[:], ctxT_d[:, :].rearrange("(o p) f -> p o f", p=P))
            nc.scalar.dma_start(wvS[:], wvT_d[:, 0:512].rearrange("(o p) d -> p o d", p=P))

            # ---------- AdaLN parameters (after xT so its DMAs lead the queue) ----------
            cond_t = cn.tile([P, 6, CT], F32, tag="cond")
            nc.sync.dma_start(cond_t[:], cond_d[:, :, :])
            ada_t = cn.tile([P, 6, CT], F32, tag="ada")
            nc.sync.dma_start(ada_t[:], ada_d[:, :, :])
            g_t = cn.tile([P, 6, CT], F32, tag="g")                 # gamma1,gamma2,scale1,scale2,shift1,shift2
            nc.vector.tensor_tensor(g_t[:], cond_t[:], ada_t[:], ALU.add)
            s1p1 = cn.tile([P, CT], F32, tag="s1p1")                # scale1 + 1
            nc.vector.tensor_scalar_add(s1p1[:], g_t[:, 2, :], 1.0)
            s2p1 = cn.tile([P, CT], F32, tag="s2p1")                # scale2 + 1
            nc.vector.tensor_scalar_add(s2p1[:], g_t[:, 3, :], 1.0)
            bo_t = cn.tile([P, CT], F32, tag="bo")
            nc.sync.dma_start(bo_t[:], bo_d[:, :])
            b1_t = cn.tile([P, ET], F32, tag="b1")
            nc.sync.dma_start(b1_t[:], b1_d[:, :])
            b2_t = cn.tile([P, CT], F32, tag="b2")
            nc.sync.dma_start(b2_t[:], b2_d[:, :])
            bog1 = cn.tile([P, CT], F32, tag="bog1")                # bo * gamma1
            nc.vector.tensor_tensor(bog1[:], bo_t[:], g_t[:, 0, :], ALU.mult)
            b2g2 = cn.tile([P, CT], F32, tag="b2g2")                # b2 * gamma2
            nc.vector.tensor_tensor(b2g2[:], b2_t[:], g_t[:, 1, :], ALU.mult)

            # ---------- V projection, d-half 0 (heads 0..7) ----------
            def v_half(half):
                for mt in range(MT):
                    v_ps = psS.tile([P, 512], F32, tag="accS")
                    for ct in range(CT):
                        nc.tensor.matmul(v_ps[:], ctxT[:, ct, mt * P:(mt + 1) * P], wvS[:, ct, :],
                                         start=(ct == 0), stop=(ct == CT - 1))
                    nc.scalar.copy(vaug[:, mt, half * 8:(half + 1) * 8, 0:HD],
                                   v_ps[:].rearrange("p (h d) -> p h d", d=HD))

            # LN1 broadcast + modulation first: Q-projection then runs on PE
            # while the context/Wv DMAs are still in flight
            bc_r1, bc_n1 = ln_bc(r1, n1)
            modx = p1.tile([P, CT, LH], BF16, tag="mod")
            ln_apply(xT, bc_r1, bc_n1, s1p1, 4, modx)

            def k_mm(dt):
                wk_st = p4.tile([P, CT, P], BF16, tag="wst")
                nc.sync.dma_start(wk_st[:], wkT_d[:, dt, :, :])
                k_ps = psS.tile([P, LC], F32, tag="accS")
                for ct in range(CT):
                    nc.tensor.matmul(k_ps[:, 0:512], wk_st[:, ct, :], ctxT[:, ct, 0:512],
                                     start=(ct == 0), stop=(ct == CT - 1))
                    nc.tensor.matmul(k_ps[:, 512:1024], wk_st[:, ct, :], ctxT[:, ct, 512:1024],
                                     start=(ct == 0), stop=(ct == CT - 1))
                return k_ps

            def k_copy(k_ps):
                kdt = p2.tile([P, LC], BF16, tag="kdt")
                nc.vector.tensor_copy(kdt[:], k_ps[:])
                return kdt

            def q_proj(dt):
                wq_st = p4.tile([P, CT, P], BF16, tag="wst")
                nc.sync.dma_start(wq_st[:], wqT_d[:, dt, :, :])
                q_ps = psS.tile([P, LH], F32, tag="accS")
                for ct in range(CT):
                    nc.tensor.matmul(q_ps[:], wq_st[:, ct, :], modx[:, ct, :],
                                     start=(ct == 0), stop=(ct == CT - 1))
                nc.scalar.mul(qT[:, dt, :], q_ps[:], SCALE)

            for dt in range(CT):
                q_proj(dt)
            v_half(0)
            # second half of Wv^T streams in while K0 runs
            wvS2 = p1.tile([P, CT, 512], BF16, tag="wvS")
            nc.scalar.dma_start(wvS2[:], wvT_d[:, 512:1024].rearrange("(o p) d -> p o d", p=P))
            kd = {0: k_copy(k_mm(0))}
            wvS = wvS2
            v_half(1)

            # ---------- attention: two-iteration software pipeline ----------
            # iteration i emits: finish(i-2) | k(i+1) | scores+softmax(i) | attnv(i-1)
            cat = p1.tile([P, CT, LH], F32R, tag="wvS")             # out^T of attention, head-concat

            def gen_scores(dt, out):
                """Yield after each pair of score matmuls (8 chunks)."""
                kcur = kd[dt]
                for hh in range(2):
                    h = 2 * dt + hh
                    probs = pp.tile([P, MT, LH], BF16, tag="probs")
                    out.append(probs)
                    for mp in range(MT // 2):           # pairs of m-tiles
                        sc = psS.tile([P, LC], F32, tag="accS")
                        for j in range(2):
                            mt = 2 * mp + j
                            nc.tensor.matmul(
                                sc[:, j * 512:(j + 1) * 512],
                                kcur[hh * HD:(hh + 1) * HD, mt * P:(mt + 1) * P],
                                qT[hh * HD:(hh + 1) * HD, dt, :],
                                start=True, stop=True)
                        # exp(s + b) = exp(s)*exp(b); host ships exp(bias)
                        bias_t = p4.tile([P, 2, LH], BF16, tag="biast")
                        nc.gpsimd.dma_start(
                            bias_t[:], biasT_d[h, 2 * mp * P:(2 * mp + 2) * P, :]
                            .rearrange("(t p) l -> p t l", p=P))
                        es = p3.tile([P, 2, LH], BF16, tag="es")
                        nc.scalar.activation(es[:], sc[:].rearrange("p (t l) -> p t l", t=2),
                                             AF.Exp, bias=0.0, scale=1.0)
                        nc.vector.tensor_tensor(probs[:, 2 * mp:2 * mp + 2, :], es[:],
                                                bias_t[:], ALU.mult)
                        yield

            def gen_k(dt):
                """Yield after every 2 K matmuls (8 chunks)."""
                wk_st = p4.tile([P, CT, P], BF16, tag="wst")
                nc.sync.dma_start(wk_st[:], wkT_d[:, dt, :, :])
                k_ps = psS.tile([P, LC], F32, tag="accS")
                kd[dt] = ("pending", k_ps)
                for ct in range(CT):
                    nc.tensor.matmul(k_ps[:, 0:512], wk_st[:, ct, :], ctxT[:, ct, 0:512],
                                     start=(ct == 0), stop=(ct == CT - 1))
                    nc.tensor.matmul(k_ps[:, 512:1024], wk_st[:, ct, :], ctxT[:, ct, 512:1024],
                                     start=(ct == 0), stop=(ct == CT - 1))
                    yield

            def gen_attnv(dt, probs2, out):
                """Yield after every 2 attn@v matmuls (8 chunks)."""
                for hh in range(2):
                    h = 2 * dt + hh
                    probs = probs2[hh]
                    av = psA.tile([P, LH], F32, tag="avp")
                    out.append((dt, hh, av))
                    for mt in range(MT):
                        nc.tensor.matmul(av[0:HD + 1, :], vaug[:, mt, h, :], probs[:, mt, :],
                                         start=(mt == 0), stop=(mt == MT - 1))
                        if mt % 2 == 1:
                            yield

            def emit_recip(pend_av):
                # all-DVE reciprocal chain, emitted at the start of the NEXT
                # iteration so it runs while PE does the K projection
                out = []
                for (dt, hh, av) in pend_av:
                    ssr = p3.tile([1, LH], F32, tag="rowtmp")
                    nc.vector.tensor_copy(ssr[:], av[HD:HD + 1, :])
                    rec = p3.tile([1, LH], F32, tag="rowtmp")
                    nc.vector.reciprocal_approx_fast(rec[:], ssr[:])
                    rec_r = p3.tile([1, LH], F32R, tag="rowtmp")
                    nc.vector.tensor_copy(rec_r[:], rec[:])
                    out.append((dt, hh, av, rec_r))
                return out

            def head_finish(pdt, phh, av, rec_r):
                bc_ps = psS.tile([P, LH], F32, tag="accS")
                nc.tensor.matmul(bc_ps[0:HD, :], ones_row[:, 0:HD], rec_r[:],
                                 start=True, stop=True)
                bc_s = p4.tile([HD, LH], F32, tag="tmpA")
                nc.vector.tensor_copy(bc_s[:], bc_ps[0:HD, :])
                if phh == 0:
                    nc.vector.tensor_tensor(cat[0:HD, pdt, :], av[0:HD, :], bc_s[:], ALU.mult)
                else:
                    tmp_o = p4.tile([HD, LH], F32R, tag="tmpB")
                    nc.vector.tensor_tensor(tmp_o[:], av[0:HD, :], bc_s[:], ALU.mult)
                    nc.sync.dma_start(cat[HD:P, pdt, :], tmp_o[:])   # partition shift

            def drain(g):
                if g is not None:
                    for _ in g:
                        pass

            probs_by_dt = {}
            pend_fin = []   # (dt, hh, av, rec_r) for head_finish one iteration later
            for i in range(CT + 2):
                sc_out = []
                av_out = []
                kp = None
                if i + 1 < CT:
                    g = gen_k(i + 1)
                    drain(g)
                    kp = kd[i + 1][1]
                for args in pend_fin:
                    head_finish(*args)              # bc + normalization for pair i-1
                pend_fin = []
                if kp is not None:
                    kd[i + 1] = k_copy(kp)          # DVE copy before the probs mults
                if i < CT:
                    drain(gen_scores(i, sc_out))
                    probs_by_dt[i] = sc_out
                if 1 <= i <= CT:
                    drain(gen_attnv(i - 1, probs_by_dt.pop(i - 1), av_out))
                pend_fin = emit_recip(av_out)       # DVE/ACT chain, runs early next iter
            for args in pend_fin:
                head_finish(*args)

            # ---------- output projection + gated residual, LN2 stats interleaved ----------
            xsum2 = psA.tile([1, LH], F32, tag="avp")
            ssum2 = psA.tile([1, LH], F32, tag="avp")
            for ct2 in range(CT):
                wo_st = p4.tile([P, CT, P], F32R, tag="wst")
                nc.sync.dma_start(wo_st[:], woT_d[:, ct2, :, :])
                ao_ps = psS.tile([P, LH], F32, tag="accS")
                for ct in range(CT):
                    nc.tensor.matmul(ao_ps[:], wo_st[:, ct, :], cat[:, ct, :],
                                     start=(ct == 0), stop=(ct == CT - 1))
                t = p4.tile([P, LH], F32, tag="tmpB")
                nc.scalar.activation(t[:], ao_ps[:], AF.Identity,
                                     bias=bog1[:, ct2:ct2 + 1], scale=g_t[:, 0, ct2:ct2 + 1])
                nc.vector.tensor_tensor(xT[:, ct2, :], t[:], xT[:, ct2, :], ALU.add)
                ln_stats_ct(xT, xsum2, ssum2, ct2)

            r2, n2 = ln_rows(xsum2, ssum2)
            bc_r2, bc_n2 = ln_bc(r2, n2)
            modf = p1.tile([P, CT, LH], BF16, tag="mod")
            ln_apply(xT, bc_r2, bc_n2, s2p1, 5, modf)

            # ---------- FFN ----------
            hT = p1.tile([P, ET, LH], BF16, tag="bigA")             # reuses ctxT slot
            for et in range(ET):
                w1_st = p4.tile([P, CT, P], BF16, tag="wst")
                (nc.sync if et % 2 == 0 else nc.gpsimd).dma_start(w1_st[:], w1T_d[:, et, :, :])
                h_ps = psS.tile([P, LH], F32, tag="accS")
                for ct in range(CT):
                    nc.tensor.matmul(h_ps[:], w1_st[:, ct, :], modf[:, ct, :],
                                     start=(ct == 0), stop=(ct == CT - 1))
                nc.scalar.activation(hT[:, et, :], h_ps[:], AF.Gelu_apprx_tanh,
                                     bias=b1_t[:, et:et + 1], scale=1.0)

            for ct2 in range(CT):
                f_ps = psS.tile([P, LH], F32, tag="accS")
                for eh in range(2):
                    w2_st = p2.tile([P, 16, P], BF16, tag="w2st")
                    (nc.sync if eh == 0 else nc.gpsimd).dma_start(w2_st[:], w2T_d[:, ct2, eh, :, :])
                    for ei in range(16):
                        et = eh * 16 + ei
                        nc.tensor.matmul(f_ps[:], w2_st[:, ei, :], hT[:, et, :],
                                         start=(et == 0), stop=(et == ET - 1))
                t = p4.tile([P, LH], F32, tag="tmpB")
                nc.scalar.activation(t[:], f_ps[:], AF.Identity,
                                     bias=b2g2[:, ct2:ct2 + 1], scale=g_t[:, 1, ct2:ct2 + 1])
                o_t = p4.tile([P, LH], F32, tag="tmpA")
                nc.vector.tensor_tensor(o_t[:], t[:], xT[:, ct2, :], ALU.add)
                nc.sync.dma_start(outT_d[ct2 * P:(ct2 + 1) * P, :], o_t[:])

    nc.compile()
    return nc


_NC = None


def _get_nc():
    global _NC
    if _NC is None:
        _NC = build()
    return _NC


def _shard(inputs):
    f32 = lambda a: np.ascontiguousarray(a, dtype=np.float32)
    bf16 = ml_dtypes.bfloat16
    x = f32(inputs["x"]); context = f32(inputs["context"])
    cond_BD = f32(inputs["cond_BD"]); attn_bias = f32(inputs["attn_bias"])
    ada_gss = f32(inputs["ada_gss"])
    Wq = f32(inputs["Wq"]); Wk = f32(inputs["Wk"]); Wv = f32(inputs["Wv"])
    Wo = f32(inputs["Wo"]); bo = f32(inputs["bo"])
    W1 = f32(inputs["W1"]); b1 = f32(inputs["b1"])
    W2 = f32(inputs["W2"]); b2 = f32(inputs["b2"])

    shared = {
        "wqT": np.ascontiguousarray(
            Wq.T.reshape(CT, P, CT, P).transpose(1, 2, 0, 3)).astype(bf16),
        "wkT": np.ascontiguousarray(
            Wk.T.reshape(CT, P, CT, P).transpose(1, 2, 0, 3)).astype(bf16),
        "wvT": np.ascontiguousarray(Wv.T).astype(bf16),
        "woT": np.ascontiguousarray(
            Wo.T.reshape(CT, P, CT, P).transpose(1, 2, 0, 3)),
        "w1T": np.ascontiguousarray(
            W1.T.reshape(CT, P, ET, P).transpose(1, 2, 0, 3)).astype(bf16),
        "w2T": np.ascontiguousarray(
            W2.T.reshape(2, 16, P, CT, P).transpose(2, 3, 0, 1, 4)).astype(bf16),
        "bo": np.ascontiguousarray(bo.reshape(CT, P).T),
        "b1": np.ascontiguousarray(b1.reshape(ET, P).T),
        "b2": np.ascontiguousarray(b2.reshape(CT, P).T),
        "ada": np.ascontiguousarray(ada_gss[0, 0].reshape(6, CT, P).transpose(2, 0, 1)),
    }
    in_maps = []
    for i in range(NCORES):
        b, lh = i // 2, i % 2
        l0 = lh * LH
        m = dict(shared)
        m["xT"] = np.ascontiguousarray(x[b, l0:l0 + LH, :].T)
        m["ctxT"] = np.ascontiguousarray(context[b].T).astype(bf16)
        m["biasT"] = np.exp(np.ascontiguousarray(
            attn_bias[b, :, l0:l0 + LH, :].transpose(0, 2, 1))).astype(bf16)
        m["cond"] = np.ascontiguousarray(cond_BD[b, 0].reshape(6, CT, P).transpose(2, 0, 1))
        in_maps.append(m)
    return in_maps


def kernel(**inputs) -> np.ndarray:
    nc = _get_nc()
    in_maps = _shard(inputs)
    res = run_bass_kernel_spmd(nc, in_maps, core_ids=list(range(NCORES)))
    out = np.empty((B, L, C), dtype=np.float32)
    for i in range(NCORES):
        b, lh = i // 2, i % 2
        out[b, lh * LH:(lh + 1) * LH, :] = res.results[i]["outT"].T
    return out


# revision 36
# speedup vs baseline: 1.1500x; 1.1500x over previous
"""AdaLN cross-attention + FFN block on 8 TRN2 NeuronCores.

Sharding: 8 cores = 4 batches x 2 L-halves (512 rows each). No collectives:
K/V projections are duplicated across the pair of cores sharing a batch
(~14% extra FLOPs), everything else splits cleanly along L.

Layout: the whole kernel runs TRANSPOSED — activations are [C, L] with the
channel dim on partitions. This makes every matmul natural (contraction dim
on partitions), makes the AdaLN scale/shift/gamma per-partition broadcasts,
and costs zero on-device transposes. The host supplies x^T, context^T,
exp(bias)^T (per-head [m, l]) and pre-transposed weights; the output comes
back as out^T and is transposed on host.

dtypes: bf16 matmul inputs for QKV/scores/attention/FFN (f32 PSUM
accumulation everywhere), float32r for the output projection, f32 for
LayerNorm statistics and residuals.

The emission order is software-pipelined (PE executes its queue in order):
the attention loop runs with a two-iteration skew — scores for head-pair i,
attention*V for pair i-1, and normalization for pair i-2 are emitted
together — so no engine ever waits on another's freshest output.
"""
import sys
if "/opt/trn_rl_repo" not in sys.path:
    sys.path.insert(0, "/opt/trn_rl_repo")

import numpy as np
import ml_dtypes

import concourse.bass as bass
import concourse.mybir as mybir
import concourse.tile as tile
from concourse import bacc
from concourse.bass_utils import run_bass_kernel_spmd

B, L, LC, C, H, HD = 4, 1024, 1024, 1024, 16, 64
P = 128
LH = 512                 # L rows per core
CT = C // P              # 8
MT = LC // P             # 8
E = 4 * C                # 4096
ET = E // P              # 32
SCALE = 0.25 / (HD ** 0.5)
EPS = 1e-5

F32 = mybir.dt.float32
F32R = mybir.dt.float32r
BF16 = mybir.dt.bfloat16
AF = mybir.ActivationFunctionType
ALU = mybir.AluOpType

NCORES = 8


def build():
    nc = bacc.Bacc("TRN2", target_bir_lowering=False, debug=False, num_devices=NCORES)

    xT_d = nc.declare_dram_parameter("xT", [C, LH], F32, isOutput=False)
    ctxT_d = nc.declare_dram_parameter("ctxT", [C, LC], BF16, isOutput=False)
    biasT_d = nc.declare_dram_parameter("biasT", [H, LC, LH], BF16, isOutput=False)
    wqT_d = nc.declare_dram_parameter("wqT", [P, CT, CT, P], BF16, isOutput=False)
    wkT_d = nc.declare_dram_parameter("wkT", [P, CT, CT, P], BF16, isOutput=False)
    wvT_d = nc.declare_dram_parameter("wvT", [C, C], BF16, isOutput=False)
    woT_d = nc.declare_dram_parameter("woT", [P, CT, CT, P], F32R, isOutput=False)
    w1T_d = nc.declare_dram_parameter("w1T", [P, ET, CT, P], BF16, isOutput=False)
    w2T_d = nc.declare_dram_parameter("w2T", [P, CT, 2, 16, P], BF16, isOutput=False)
    bo_d = nc.declare_dram_parameter("bo", [P, CT], F32, isOutput=False)
    b1_d = nc.declare_dram_parameter("b1", [P, ET], F32, isOutput=False)
    b2_d = nc.declare_dram_parameter("b2", [P, CT], F32, isOutput=False)
    cond_d = nc.declare_dram_parameter("cond", [P, 6, CT], F32, isOutput=False)
    ada_d = nc.declare_dram_parameter("ada", [P, 6, CT], F32, isOutput=False)
    outT_d = nc.declare_dram_parameter("outT", [C, LH], F32, isOutput=True)

    with tile.TileContext(nc) as tc:
        with (
            tc.tile_pool(name="cn", bufs=1) as cn,
            tc.tile_pool(name="p1", bufs=1) as p1,
            tc.tile_pool(name="p2", bufs=2) as p2,
            tc.tile_pool(name="p3", bufs=3) as p3,
            tc.tile_pool(name="p4", bufs=4) as p4,
            tc.tile_pool(name="pp", bufs=4) as pp,
            tc.tile_pool(name="rw2", bufs=2) as rw2,
            tc.tile_pool(name="psS", bufs=3, space="PSUM") as psS,      # [P,1024] universal
            tc.tile_pool(name="psA", bufs=2, space="PSUM") as psA,      # [P,512] av/stats/bcast
        ):
            # ---------- constants / params ----------
            ones_f = cn.tile([P, 1], F32, tag="ones_f")
            nc.vector.memset(ones_f[:], 1.0)
            ones_col = cn.tile([P, 1], BF16, tag="ones_col")        # lhsT [K=P, M=1]
            nc.scalar.copy(ones_col[:], ones_f[:])
            ones_col32 = cn.tile([P, 1], F32, tag="ones_col32")
            nc.vector.tensor_copy(ones_col32[:], ones_f[:])
            ones_rf = cn.tile([1, P], F32, tag="ones_rf")
            nc.vector.memset(ones_rf[:], 1.0)
            ones_row = cn.tile([1, P], F32R, tag="ones_row")        # lhsT [K=1, M=P]
            nc.scalar.copy(ones_row[:], ones_rf[:])
            eps_t = cn.tile([1, 1], F32, tag="eps")
            nc.vector.memset(eps_t[:], EPS)

            # ---------- big persistent tensors ----------
            ctxT = p1.tile([P, CT, LC], BF16, tag="bigA")           # shared slot with hT
            wvS = p1.tile([P, CT, 512], BF16, tag="wvS")            # half of Wv^T
            xT = p1.tile([P, CT, LH], F32, tag="xT")
            qT = p1.tile([P, CT, LH], BF16, tag="qT")
            vaug = p1.tile([P, MT, H, HD + 1], BF16, tag="vaug")
            nc.vector.memset(vaug[:, :, :, HD:HD + 1], 1.0)         # softmax-denominator ones column

            # ---------- LN pieces ----------
            def ln_stats_ct(src, xsum, ssum, ct, f32_stats=False):
                if f32_stats:
                    # direct f32 matmuls (4 cyc/row) — PE is idle here anyway and
                    # this kills the DMA->cast->matmul staircase at startup
                    sq = p2.tile([P, LH], F32, tag="kdt")
                    nc.scalar.activation(sq[:], src[:, ct, :], AF.Square, bias=0.0, scale=1.0)
                    nc.tensor.matmul(xsum[:], ones_col32[:], src[:, ct, :],
                                     start=(ct == 0), stop=(ct == CT - 1))
                    nc.tensor.matmul(ssum[:], ones_col32[:], sq[:],
                                     start=(ct == 0), stop=(ct == CT - 1))
                    return
                xr = p4.tile([P, LH], BF16, tag="tmpA")
                nc.vector.tensor_copy(xr[:], src[:, ct, :])
                sq = p4.tile([P, LH], BF16, tag="tmpB")
                nc.scalar.activation(sq[:], src[:, ct, :], AF.Square, bias=0.0, scale=1.0)
                nc.tensor.matmul(xsum[:], ones_col[:], xr[:], start=(ct == 0), stop=(ct == CT - 1))
                nc.tensor.matmul(ssum[:], ones_col[:], sq[:], start=(ct == 0), stop=(ct == CT - 1))

            def ln_rows(xsum, ssum):
                mu = cn.tile([1, LH], F32, tag="mu")
                nc.vector.tensor_scalar_mul(mu[:], xsum[:], 1.0 / C)
                ex2 = p3.tile([1, LH], F32, tag="rowtmp")
                nc.vector.tensor_scalar_mul(ex2[:], ssum[:], 1.0 / C)
                mu2 = p3.tile([1, LH], F32, tag="rowtmp")
                nc.vector.tensor_tensor(mu2[:], mu[:], mu[:], ALU.mult)
                var = p3.tile([1, LH], F32, tag="rowtmp")
                nc.vector.tensor_tensor(var[:], ex2[:], mu2[:], ALU.subtract)
                sd = p3.tile([1, LH], F32, tag="rowtmp")
                nc.scalar.activation(sd[:], var[:], AF.Sqrt, bias=eps_t[:, 0:1], scale=1.0)
                rstd = cn.tile([1, LH], F32, tag="rstd")
                nc.vector.reciprocal_approx_fast(rstd[:], sd[:])
                rstd_r = cn.tile([1, LH], F32R, tag="rstd_r")
                nc.scalar.copy(rstd_r[:], rstd[:])
                nmr = p3.tile([1, LH], F32, tag="rowtmp")
                nc.vector.tensor_tensor(nmr[:], mu[:], rstd[:], ALU.mult)
                nmr_r = cn.tile([1, LH], F32R, tag="nmr_r")
                nc.scalar.mul(nmr_r[:], nmr[:], -1.0)               # -(mu*rstd)
                return rstd_r, nmr_r

            def ln_bc(rstd_r, nmr_r):
                # broadcast rows across partitions via K=1 matmul, then park in
                # SBUF so the PSUM banks free up immediately
                bc_rp = psA.tile([P, LH], F32, tag="avp")
                nc.tensor.matmul(bc_rp[:], ones_row[:], rstd_r[:], start=True, stop=True)
                bc_r = rw2.tile([P, LH], F32, tag="bcs")
                nc.scalar.copy(bc_r[:], bc_rp[:])
                bc_np = psA.tile([P, LH], F32, tag="avp")
                nc.tensor.matmul(bc_np[:], ones_row[:], nmr_r[:], start=True, stop=True)
                bc_n = rw2.tile([P, LH], F32, tag="bcs")
                nc.scalar.copy(bc_n[:], bc_np[:])
                return bc_r, bc_n

            def ln_apply(src, bc_r, bc_n, sc_col, sh_idx, out_mod):
                for ct in range(CT):
                    t1 = p4.tile([P, LH], F32, tag="tmpA")
                    nc.vector.tensor_tensor(t1[:], src[:, ct, :], bc_r[:], ALU.mult)
                    t2 = p4.tile([P, LH], F32, tag="tmpB")
                    nc.vector.tensor_tensor(t2[:], t1[:], bc_n[:], ALU.add)
                    nc.scalar.activation(out_mod[:, ct, :], t2[:], AF.Identity,
                                         bias=g_t[:, sh_idx, ct:ct + 1],
                                         scale=sc_col[:, ct:ct + 1])

            # LN1 stats: xT arrives per-c-tile so the first matmuls start early
            xsum1 = psA.tile([1, LH], F32, tag="avp")
            ssum1 = psA.tile([1, LH], F32, tag="avp")
            for ct in range(CT):
                nc.sync.dma_start(xT[:, ct, :], xT_d[ct * P:(ct + 1) * P, :])
                ln_stats_ct(xT, xsum1, ssum1, ct)
            r1, n1 = ln_rows(xsum1, ssum1)
            # context + Wv stream in AFTER xT so LN1 isn't starved of HBM bandwidth
            nc.gpsimd.dma_start(ctxT[:], ctxT_d[:, :].rearrange("(o p) f -> p o f", p=P))
            nc.scalar.dma_start(wvS[:], wvT_d[:, 0:512].rearrange("(o p) d -> p o d", p=P))

            # ---------- AdaLN parameters (after xT so its DMAs lead the queue) ----------
            cond_t = cn.tile([P, 6, CT], F32, tag="cond")
            nc.sync.dma_start(cond_t[:], cond_d[:, :, :])
            ada_t = cn.tile([P, 6, CT], F32, tag="ada")
            nc.sync.dma_start(ada_t[:], ada_d[:, :, :])
            g_t = cn.tile([P, 6, CT], F32, tag="g")                 # gamma1,gamma2,scale1,scale2,shift1,shift2
            nc.vector.tensor_tensor(g_t[:], cond_t[:], ada_t[:], ALU.add)
            s1p1 = cn.tile([P, CT], F32, tag="s1p1")                # scale1 + 1
            nc.vector.tensor_scalar_add(s1p1[:], g_t[:, 2, :], 1.0)
            s2p1 = cn.tile([P, CT], F32, tag="s2p1")                # scale2 + 1
            nc.vector.tensor_scalar_add(s2p1[:], g_t[:, 3, :], 1.0)
            bo_t = cn.tile([P, CT], F32, tag="bo")
            nc.sync.dma_start(bo_t[:], bo_d[:, :])
            b1_t = cn.tile([P, ET], F32, tag="b1")
            nc.sync.dma_start(b1_t[:], b1_d[:, :])
            b2_t = cn.tile([P, CT], F32, tag="b2")
            nc.sync.dma_start(b2_t[:], b2_d[:, :])
            bog1 = cn.tile([P, CT], F32, tag="bog1")                # bo * gamma1
            nc.vector.tensor_tensor(bog1[:], bo_t[:], g_t[:, 0, :], ALU.mult)
            b2g2 = cn.tile([P, CT], F32, tag="b2g2")                # b2 * gamma2
            nc.vector.tensor_tensor(b2g2[:], b2_t[:], g_t[:, 1, :], ALU.mult)

            # ---------- V projection, d-half 0 (heads 0..7) ----------
            def v_half(half):
                for mt in range(MT):
                    v_ps = psS.tile([P, 512], F32, tag="accS")
                    for ct in range(CT):
                        nc.tensor.matmul(v_ps[:], ctxT[:, ct, mt * P:(mt + 1) * P], wvS[:, ct, :],
                                         start=(ct == 0), stop=(ct == CT - 1))
                    nc.scalar.copy(vaug[:, mt, half * 8:(half + 1) * 8, 0:HD],
                                   v_ps[:].rearrange("p (h d) -> p h d", d=HD))

            # LN1 broadcast + modulation first: Q-projection then runs on PE
            # while the context/Wv DMAs are still in flight
            bc_r1, bc_n1 = ln_bc(r1, n1)
            modx = p1.tile([P, CT, LH], BF16, tag="mod")
            ln_apply(xT, bc_r1, bc_n1, s1p1, 4, modx)

            def k_mm(dt):
                wk_st = p4.tile([P, CT, P], BF16, tag="wst")
                nc.sync.dma_start(wk_st[:], wkT_d[:, dt, :, :])
                k_ps = psS.tile([P, LC], F32, tag="accS")
                for ct in range(CT):
                    nc.tensor.matmul(k_ps[:, 0:512], wk_st[:, ct, :], ctxT[:, ct, 0:512],
                                     start=(ct == 0), stop=(ct == CT - 1))
                    nc.tensor.matmul(k_ps[:, 512:1024], wk_st[:, ct, :], ctxT[:, ct, 512:1024],
                                     start=(ct == 0), stop=(ct == CT - 1))
                return k_ps

            def k_copy(k_ps):
                kdt = p2.tile([P, LC], BF16, tag="kdt")
                nc.vector.tensor_copy(kdt[:], k_ps[:])
                return kdt

            def q_proj(dt):
                wq_st = p4.tile([P, CT, P], BF16, tag="wst")
                nc.sync.dma_start(wq_st[:], wqT_d[:, dt, :, :])
                q_ps = psS.tile([P, LH], F32, tag="accS")
                for ct in range(CT):
                    nc.tensor.matmul(q_ps[:], wq_st[:, ct, :], modx[:, ct, :],
                                     start=(ct == 0), stop=(ct == CT - 1))
                nc.scalar.mul(qT[:, dt, :], q_ps[:], SCALE)

            for dt in range(CT):
                q_proj(dt)
            v_half(0)
            # second half of Wv^T streams in while K0 runs
            wvS2 = p1.tile([P, CT, 512], BF16, tag="wvS")
            nc.scalar.dma_start(wvS2[:], wvT_d[:, 512:1024].rearrange("(o p) d -> p o d", p=P))
            kd = {0: k_copy(k_mm(0))}
            wvS = wvS2
            v_half(1)

            # ---------- attention: two-iteration software pipeline ----------
            # iteration i emits: finish(i-2) | k(i+1) | scores+softmax(i) | attnv(i-1)
            cat = p1.tile([P, CT, LH], F32R, tag="wvS")             # out^T of attention, head-concat

            def gen_scores(dt, out):
                """Yield after each pair of score matmuls (8 chunks)."""
                kcur = kd[dt]
                for hh in range(2):
                    h = 2 * dt + hh
                    probs = pp.tile([P, MT, LH], BF16, tag="probs")
                    out.append(probs)
                    for mp in range(MT // 2):           # pairs of m-tiles
                        sc = psS.tile([P, LC], F32, tag="accS")
                        for j in range(2):
                            mt = 2 * mp + j
                            nc.tensor.matmul(
                                sc[:, j * 512:(j + 1) * 512],
                                kcur[hh * HD:(hh + 1) * HD, mt * P:(mt + 1) * P],
                                qT[hh * HD:(hh + 1) * HD, dt, :],
                                start=True, stop=True)
                        # exp(s + b) = exp(s)*exp(b); host ships exp(bias)
                        bias_t = p4.tile([P, 2, LH], BF16, tag="biast")
                        nc.gpsimd.dma_start(
                            bias_t[:], biasT_d[h, 2 * mp * P:(2 * mp + 2) * P, :]
                            .rearrange("(t p) l -> p t l", p=P))
                        es = p3.tile([P, 2, LH], BF16, tag="es")
                        nc.scalar.activation(es[:], sc[:].rearrange("p (t l) -> p t l", t=2),
                                             AF.Exp, bias=0.0, scale=1.0)
                        nc.vector.tensor_tensor(probs[:, 2 * mp:2 * mp + 2, :], es[:],
                                                bias_t[:], ALU.mult)
                        yield

            def gen_k(dt):
                """Yield after every 2 K matmuls (8 chunks)."""
                wk_st = p4.tile([P, CT, P], BF16, tag="wst")
                nc.sync.dma_start(wk_st[:], wkT_d[:, dt, :, :])
                k_ps = psS.tile([P, LC], F32, tag="accS")
                kd[dt] = ("pending", k_ps)
                for ct in range(CT):
                    nc.tensor.matmul(k_ps[:, 0:512], wk_st[:, ct, :], ctxT[:, ct, 0:512],
                                     start=(ct == 0), stop=(ct == CT - 1))
                    nc.tensor.matmul(k_ps[:, 512:1024], wk_st[:, ct, :], ctxT[:, ct, 512:1024],
                                     start=(ct == 0), stop=(ct == CT - 1))
                    yield

            def gen_attnv(dt, probs2, out):
                """Yield after every 2 attn@v matmuls (8 chunks)."""
                for hh in range(2):
                    h = 2 * dt + hh
                    probs = probs2[hh]
                    av = psA.tile([P, LH], F32, tag="avp")
                    out.append((dt, hh, av))
                    for mt in range(MT):
                        nc.tensor.matmul(av[0:HD + 1, :], vaug[:, mt, h, :], probs[:, mt, :],
                                         start=(mt == 0), stop=(mt == MT - 1))
                        if mt % 2 == 1:
                            yield

            def emit_recip(pend_av):
                # all-DVE reciprocal chain, emitted at the start of the NEXT
                # iteration so it runs while PE does the K projection
                out = []
                for (dt, hh, av) in pend_av:
                    ssr = p3.tile([1, LH], F32, tag="rowtmp")
                    nc.scalar.copy(ssr[:], av[HD:HD + 1, :])
                    rec = p3.tile([1, LH], F32, tag="rowtmp")
                    nc.vector.reciprocal_approx_fast(rec[:], ssr[:])
                    rec_r = p3.tile([1, LH], F32R, tag="rowtmp")
                    nc.scalar.copy(rec_r[:], rec[:])
                    out.append((dt, hh, av, rec_r))
                return out

            def head_finish(pdt, phh, av, rec_r):
                bc_ps = psS.tile([P, LH], F32, tag="accS")
                nc.tensor.matmul(bc_ps[0:HD, :], ones_row[:, 0:HD], rec_r[:],
                                 start=True, stop=True)
                bc_s = p4.tile([HD, LH], F32, tag="tmpA")
                nc.scalar.copy(bc_s[:], bc_ps[0:HD, :])
                if phh == 0:
                    nc.vector.tensor_tensor(cat[0:HD, pdt, :], av[0:HD, :], bc_s[:], ALU.mult)
                else:
                    tmp_o = p4.tile([HD, LH], F32R, tag="tmpB")
                    nc.vector.tensor_tensor(tmp_o[:], av[0:HD, :], bc_s[:], ALU.mult)
                    nc.sync.dma_start(cat[HD:P, pdt, :], tmp_o[:])   # partition shift

            def drain(g):
                if g is not None:
                    for _ in g:
                        pass

            probs_by_dt = {}
            pend_fin = []   # (dt, hh, av, rec_r) for head_finish one iteration later
            for i in range(CT + 2):
                sc_out = []
                av_out = []
                kp = None
                if i + 1 < CT:
                    g = gen_k(i + 1)
                    drain(g)
                    kp = kd[i + 1][1]
                for args in pend_fin:
                    head_finish(*args)              # bc + normalization for pair i-1
                pend_fin = []
                if kp is not None:
                    kd[i + 1] = k_copy(kp)          # DVE copy before the probs mults
                if i < CT:
                    drain(gen_scores(i, sc_out))
                    probs_by_dt[i] = sc_out
                if 1 <= i <= CT:
                    drain(gen_attnv(i - 1, probs_by_dt.pop(i - 1), av_out))
                pend_fin = emit_recip(av_out)       # DVE/ACT chain, runs early next iter
            for args in pend_fin:
                head_finish(*args)

            # ---------- output projection + gated residual, LN2 stats interleaved ----------
            xsum2 = psA.tile([1, LH], F32, tag="avp")
            ssum2 = psA.tile([1, LH], F32, tag="avp")
            for ct2 in range(CT):
                wo_st = p4.tile([P, CT, P], F32R, tag="wst")
                nc.sync.dma_start(wo_st[:], woT_d[:, ct2, :, :])
                ao_ps = psS.tile([P, LH], F32, tag="accS")
                for ct in range(CT):
                    nc.tensor.matmul(ao_ps[:], wo_st[:, ct, :], cat[:, ct, :],
                                     start=(ct == 0), stop=(ct == CT - 1))
                t = p4.tile([P, LH], F32, tag="tmpB")
                nc.scalar.activation(t[:], ao_ps[:], AF.Identity,
                                     bias=bog1[:, ct2:ct2 + 1], scale=g_t[:, 0, ct2:ct2 + 1])
                nc.vector.tensor_tensor(xT[:, ct2, :], t[:], xT[:, ct2, :], ALU.add)
                ln_stats_ct(xT, xsum2, ssum2, ct2)

            r2, n2 = ln_rows(xsum2, ssum2)
            bc_r2, bc_n2 = ln_bc(r2, n2)
            modf = p1.tile([P, CT, LH], BF16, tag="mod")
            ln_apply(xT, bc_r2, bc_n2, s2p1, 5, modf)

            # ---------- FFN ----------
            hT = p1.tile([P, ET, LH], BF16, tag="bigA")             # reuses ctxT slot
            for et in range(ET):
                w1_st = p4.tile([P, CT, P], BF16, tag="wst")
                (nc.sync if et % 2 == 0 else nc.gpsimd).dma_start(w1_st[:], w1T_d[:, et, :, :])
                h_ps = psS.tile([P, LH], F32, tag="accS")
                for ct in range(CT):
                    nc.tensor.matmul(h_ps[:], w1_st[:, ct, :], modf[:, ct, :],
                                     start=(ct == 0), stop=(ct == CT - 1))
                nc.scalar.activation(hT[:, et, :], h_ps[:], AF.Gelu_apprx_tanh,
                                     bias=b1_t[:, et:et + 1], scale=1.0)

            for ct2 in range(CT):
                f_ps = psS.tile([P, LH], F32, tag="accS")
                for eh in range(2):
                    w2_st = p2.tile([P, 16, P], BF16, tag="w2st")
                    (nc.sync if eh == 0 else nc.gpsimd).dma_start(w2_st[:], w2T_d[:, ct2, eh, :, :])
                    for ei in range(16):
                        et = eh * 16 + ei
                        nc.tensor.matmul(f_ps[:], w2_st[:, ei, :], hT[:, et, :],
                                         start=(et == 0), stop=(et == ET - 1))
                t = p4.tile([P, LH], F32, tag="tmpB")
                nc.scalar.activation(t[:], f_ps[:], AF.Identity,
                                     bias=b2g2[:, ct2:ct2 + 1], scale=g_t[:, 1, ct2:ct2 + 1])
                o_t = p4.tile([P, LH], F32, tag="tmpA")
                nc.vector.tensor_tensor(o_t[:], t[:], xT[:, ct2, :], ALU.add)
                nc.sync.dma_start(outT_d[ct2 * P:(ct2 + 1) * P, :], o_t[:])

    nc.compile()
    return nc


_NC = None


def _get_nc():
    global _NC
    if _NC is None:
        _NC = build()
    return _NC


def _shard(inputs):
    f32 = lambda a: np.ascontiguousarray(a, dtype=np.float32)
    bf16 = ml_dtypes.bfloat16
    x = f32(inputs["x"]); context = f32(inputs["context"])
    cond_BD = f32(inputs["cond_BD"]); attn_bias = f32(inputs["attn_bias"])
    ada_gss = f32(inputs["ada_gss"])
    Wq = f32(inputs["Wq"]); Wk = f32(inputs["Wk"]); Wv = f32(inputs["Wv"])
    Wo = f32(inputs["Wo"]); bo = f32(inputs["bo"])
    W1 = f32(inputs["W1"]); b1 = f32(inputs["b1"])
    W2 = f32(inputs["W2"]); b2 = f32(inputs["b2"])

    shared = {
        "wqT": np.ascontiguousarray(
            Wq.T.reshape(CT, P, CT, P).transpose(1, 2, 0, 3)).astype(bf16),
        "wkT": np.ascontiguousarray(
            Wk.T.reshape(CT, P, CT, P).transpose(1, 2, 0, 3)).astype(bf16),
        "wvT": np.ascontiguousarray(Wv.T).astype(bf16),
        "woT": np.ascontiguousarray(
            Wo.T.reshape(CT, P, CT, P).transpose(1, 2, 0, 3)),
        "w1T": np.ascontiguousarray(
            W1.T.reshape(CT, P, ET, P).transpose(1, 2, 0, 3)).astype(bf16),
        "w2T": np.ascontiguousarray(
            W2.T.reshape(2, 16, P, CT, P).transpose(2, 3, 0, 1, 4)).astype(bf16),
        "bo": np.ascontiguousarray(bo.reshape(CT, P).T),
        "b1": np.ascontiguousarray(b1.reshape(ET, P).T),
        "b2": np.ascontiguousarray(b2.reshape(CT, P).T),
        "ada": np.ascontiguousarray(ada_gss[0, 0].reshape(6, CT, P).transpose(2, 0, 1)),
    }
    in_maps = []
    for i in range(NCORES):
        b, lh = i // 2, i % 2
        l0 = lh * LH
        m = dict(shared)
        m["xT"] = np.ascontiguousarray(x[b, l0:l0 + LH, :].T)
        m["ctxT"] = np.ascontiguousarray(context[b].T).astype(bf16)
        m["biasT"] = np.exp(np.ascontiguousarray(
            attn_bias[b, :, l0:l0 + LH, :].transpose(0, 2, 1))).astype(bf16)
        m["cond"] = np.ascontiguousarray(cond_BD[b, 0].reshape(6, CT, P).transpose(2, 0, 1))
        in_maps.append(m)
    return in_maps


def kernel(**inputs) -> np.ndarray:
    nc = _get_nc()
    in_maps = _shard(inputs)
    res = run_bass_kernel_spmd(nc, in_maps, core_ids=list(range(NCORES)))
    out = np.empty((B, L, C), dtype=np.float32)
    for i in range(NCORES):
        b, lh = i // 2, i % 2
        out[b, lh * LH:(lh + 1) * LH, :] = res.results[i]["outT"].T
    return out
